# revision 7
# baseline (speedup 1.0000x reference)
"""Trainium2 Bass kernel for nn_MHA_29008209117536.

MHA with a temporal-bias MLP:
  q = (Xq Wq) split-heads; k/v from Xk; scores = qk^T/8 + bias(T); softmax; out = (attn v) Wp

Key observations baked into the kernel:
  * The temporal-bias MLP collapses: t = 1/log(e+T) > 0 always, so
    leaky_relu(t * Wt1) = t * (Wt1 if Wt1>=0 else 0.2*Wt1) elementwise, and
    bias = C * t with scalar C = sum(Wt2 * leaky(Wt1)).  Computed on host.
  * Sharding: data-parallel over batch. B=8 == n_cores; core b handles batch b.
  * All matmuls run in a "T-space" layout that needs NO on-device transposes:
      - host passes XqT, XkT (the [D,S] transposes; free on host)
      - QT = Wq^T Xq^T and KT likewise:  lhsT=Wq-tile, rhs=XqT  (contract d_in)
      - V natural [S,D]:                 lhsT=XkT-tile, rhs=Wv
      - ST_h = K_h Q_h^T ([s_k, s_q]):   lhsT=KT_h, rhs=QT_h    (contract d_k)
      - softmax runs over s_k (partition dim): plain exp (no max-sub; scores
        are O(10) so fp32 exp is exact enough), denominators come free from a
        ones-column appended to V in the AV matmul.
      - OT_h = V_h^T P_h^T ([dk, s_q]):  lhsT=[V_h|1], rhs=exp(ST_h)
      - out  = OT^T Wp ([s_q, d]):       lhsT=OT_h, rhs=Wp_h    (contract dk)
  * exp(scale*x) activation folds constants: host scales Wq by 1/(8C) so that
    one Exp(scale=C) activation applies both the 1/sqrt(dk) scale and the
    additive bias (via exp(ST+Cb) = exp(C*ST') * exp(C*b), EB precomputed).
  * float32r matmul dtype: 1 cycle/row at N=512 (4x faster than fp32).
"""

import numpy as np

import concourse.bass as bass
import concourse.mybir as mybir
import concourse.tile as tile
from concourse import bacc
from concourse.bass_utils import run_bass_kernel_spmd

F32 = mybir.dt.float32
F32R = mybir.dt.float32r
AF = mybir.ActivationFunctionType

B, S, D, H, TB = 8, 512, 512, 8, 64
DK = D // H          # 64
P = 128              # partitions
NT = S // P          # 4 tiles of 128 along any 512 dim
N_CORES = 8


def build_nc(C: float, use_bias: bool):
    nc = bacc.Bacc("TRN2", target_bir_lowering=False, debug=False,
                   num_devices=N_CORES)

    xqT = nc.dram_tensor("xqT", [D, S], F32, kind="ExternalInput").ap()
    xkT = nc.dram_tensor("xkT", [D, S], F32, kind="ExternalInput").ap()
    tT = nc.dram_tensor("tT", [S, S], F32, kind="ExternalInput").ap()
    wq = nc.dram_tensor("wq", [D, D], F32, kind="ExternalInput").ap()
    wk = nc.dram_tensor("wk", [D, D], F32, kind="ExternalInput").ap()
    wv = nc.dram_tensor("wv", [D, D], F32, kind="ExternalInput").ap()
    wp = nc.dram_tensor("wp", [D, D], F32, kind="ExternalInput").ap()
    out = nc.dram_tensor("out", [S, D], F32, kind="ExternalOutput").ap()

    # [din, dout] -> [p, kt, dout]: partition = din within k-tile
    wq_t = wq.bitcast(F32R).rearrange("(kt p) d -> p kt d", p=P)
    wk_t = wk.bitcast(F32R).rearrange("(kt p) d -> p kt d", p=P)
    wv_t = wv.bitcast(F32R).rearrange("(kt p) d -> p kt d", p=P)
    # [din, dout] -> [p(64), h, dout]: per-head slice at base partition 0
    wp_t = wp.bitcast(F32R).rearrange("(h p) d -> p h d", p=DK)
    xqT_t = xqT.bitcast(F32R).rearrange("(kt p) s -> p kt s", p=P)
    xkT_t = xkT.bitcast(F32R).rearrange("(kt p) s -> p kt s", p=P)
    tT_t = tT.rearrange("(kt p) s -> p kt s", p=P)
    out_t = out.rearrange("(st p) d -> p st d", p=P)

    with tile.TileContext(nc) as tc:
        with (
            tc.tile_pool(name="const", bufs=1) as cpool,
            tc.tile_pool(name="work", bufs=2) as wpool,
            tc.tile_pool(name="otn", bufs=H) as opool,
            tc.tile_pool(name="dram", bufs=1, space="DRAM") as dpool,
            tc.tile_pool(name="pj", bufs=2, space="PSUM") as pjp,
            tc.tile_pool(name="st", bufs=1, space="PSUM") as stp,
            tc.tile_pool(name="av", bufs=2, space="PSUM") as avp,
        ):
            wq_sb = cpool.tile([P, NT, D], F32R, tag="wq")
            xq_sb = cpool.tile([P, NT, S], F32R, tag="xq")
            wk_sb = cpool.tile([P, NT, D], F32R, tag="wk")
            xk_sb = cpool.tile([P, NT, S], F32R, tag="xk")
            wv_sb = cpool.tile([P, NT, D], F32R, tag="wv")
            wp_sb = cpool.tile([DK, H, D], F32R, tag="wp")
            qt_sb = cpool.tile([P, NT, S], F32R, tag="qt")
            kt_sb = cpool.tile([P, NT, S], F32R, tag="kt")
            vb_sb = cpool.tile([P, NT, H * (DK + 1)], F32R, tag="vb")
            rb_sb = cpool.tile([DK, H, S], F32, tag="rb")
            if use_bias:
                tt_sb = cpool.tile([P, NT, S], F32, tag="tt")
                u_sb = cpool.tile([P, NT, S], F32, tag="u")
                eb_sb = cpool.tile([P, NT, S], F32, tag="eb")
            rscr = dpool.tile([H, S], F32, tag="rscr")

            # ---- input DMAs, ordered by when compute needs them ----
            for kt in range(NT):
                nc.sync.dma_start(out=wq_sb[:, kt, :], in_=wq_t[:, kt, :])
                nc.sync.dma_start(out=xq_sb[:, kt, :], in_=xqT_t[:, kt, :])
            if use_bias:
                for kt in range(NT):
                    nc.sync.dma_start(out=tt_sb[:, kt, :], in_=tT_t[:, kt, :])
            for kt in range(NT):
                nc.sync.dma_start(out=wk_sb[:, kt, :], in_=wk_t[:, kt, :])
                nc.sync.dma_start(out=xk_sb[:, kt, :], in_=xkT_t[:, kt, :])
                nc.sync.dma_start(out=wv_sb[:, kt, :], in_=wv_t[:, kt, :])
            nc.sync.dma_start(out=wp_sb, in_=wp_t)

            # ---- temporal bias: EB = exp(C / ln(e + T^T)) ----
            if use_bias:
                e_sb = cpool.tile([P, 1], F32, tag="econst")
                nc.vector.memset(e_sb, float(np.e))
                nc.scalar.activation(out=u_sb, in_=tt_sb, func=AF.Ln,
                                     bias=e_sb)
                nc.vector.reciprocal(out=u_sb, in_=u_sb)
                nc.scalar.activation(out=eb_sb, in_=u_sb, func=AF.Exp,
                                     scale=float(C))

            # ones columns of the [V | 1] blocks (memset can't emit f32r;
            # bounce through an f32 tile and let the DVE copy convert)
            vb_heads = vb_sb.rearrange("p kt (h c) -> p kt h c", c=DK + 1)
            one_sb = cpool.tile([P, NT, H, 1], F32, tag="ones")
            nc.vector.memset(one_sb, 1.0)
            nc.vector.tensor_copy(out=vb_heads[:, :, :, DK:DK + 1], in_=one_sb)

            # ---- projections: QT, KT (T-space), V (natural) ----
            for m in range(NT):
                ps = pjp.tile([P, S], F32, tag="pj")
                for kt in range(NT):
                    nc.tensor.matmul(ps,
                                     (wq_sb[:, kt, m * P:(m + 1) * P]),
                                     (xq_sb[:, kt, :]),
                                     start=(kt == 0), stop=(kt == NT - 1))
                nc.vector.tensor_copy(out=qt_sb[:, m, :], in_=ps)
            for m in range(NT):
                ps = pjp.tile([P, S], F32, tag="pj")
                for kt in range(NT):
                    nc.tensor.matmul(ps,
                                     (wk_sb[:, kt, m * P:(m + 1) * P]),
                                     (xk_sb[:, kt, :]),
                                     start=(kt == 0), stop=(kt == NT - 1))
                nc.vector.tensor_copy(out=kt_sb[:, m, :], in_=ps)
            for sv in range(NT):
                ps = pjp.tile([P, S], F32, tag="pj")
                for kt in range(NT):
                    nc.tensor.matmul(ps,
                                     (xk_sb[:, kt, sv * P:(sv + 1) * P]),
                                     (wv_sb[:, kt, :]),
                                     start=(kt == 0), stop=(kt == NT - 1))
                # scatter dk-columns into the [V | 1] per-head blocks
                nc.vector.tensor_copy(
                    out=vb_heads[:, sv, :, 0:DK],
                    in_=ps.rearrange("p (h c) -> p h c", c=DK))

            # ---- per-head attention ----
            otns = []
            for h in range(H):
                hm, hp = h // 2, (h % 2) * DK
                stw = stp.tile([P, NT, S], F32, tag="st")
                for j in range(NT):
                    nc.tensor.matmul(
                        stw[:, j, :],
                        (kt_sb[hp:hp + DK, hm, j * P:(j + 1) * P]),
                        (qt_sb[hp:hp + DK, hm, :]),
                        start=True, stop=True)
                if use_bias:
                    er = wpool.tile([P, NT, S], F32, tag="er")
                    nc.scalar.activation(out=er, in_=stw, func=AF.Exp,
                                         scale=float(C))
                    pt = wpool.tile([P, NT, S], F32R, tag="pt")
                    nc.vector.tensor_mul(out=pt, in0=er, in1=eb_sb)
                else:
                    pt = wpool.tile([P, NT, S], F32R, tag="pt")
                    nc.scalar.activation(out=pt, in_=stw, func=AF.Exp,
                                         scale=1.0)
                av = avp.tile([DK + 1, S], F32, tag="av")
                for kt in range(NT):
                    nc.tensor.matmul(
                        av,
                        (vb_sb[:, kt, h * (DK + 1):(h + 1) * (DK + 1)]),
                        (pt[:, kt, :]),
                        start=(kt == 0), stop=(kt == NT - 1))
                otu = wpool.tile([DK + 1, S], F32, tag="otu")
                nc.vector.tensor_copy(out=otu, in_=av)
                # reciprocal of the softmax sums (row DK), broadcast to DK
                # partitions via a DRAM bounce
                nc.vector.reciprocal(out=otu[DK:DK + 1, :],
                                     in_=otu[DK:DK + 1, :])
                nc.sync.dma_start(out=rscr[h:h + 1, :], in_=otu[DK:DK + 1, :])
                nc.sync.dma_start(out=rb_sb[:, h, :],
                                  in_=rscr[h:h + 1, :].to_broadcast((DK, S)))
                otn = opool.tile([DK, S], F32R, tag="otn")
                nc.vector.tensor_mul(out=otn, in0=otu[0:DK, :],
                                     in1=rb_sb[:, h, :])
                otns.append(otn)

            # ---- output projection: out[s,f] = sum_h OT_h^T Wp_h ----
            ow = []
            for st_ in range(NT):
                ps = pjp.tile([P, S], F32, tag="pj")
                for h in range(H):
                    nc.tensor.matmul(ps,
                                     (otns[h][:, st_ * P:(st_ + 1) * P]),
                                     (wp_sb[:, h, :]),
                                     start=(h == 0), stop=(h == H - 1))
                osb = wpool.tile([P, S], F32, tag="osb")
                nc.vector.tensor_copy(out=osb, in_=ps)
                ow.append(osb)
            for st_ in range(NT):
                nc.sync.dma_start(out=out_t[:, st_, :], in_=ow[st_])

    nc.compile()
    return nc


_CACHE = {}


def _get_nc(C: float, use_bias: bool):
    key = (round(C, 12), use_bias)
    if key not in _CACHE:
        _CACHE[key] = build_nc(C, use_bias)
    return _CACHE[key]


def prepare(inputs: dict):
    q = np.ascontiguousarray(np.asarray(inputs["query_input"], dtype=np.float32))
    k = np.ascontiguousarray(np.asarray(inputs["key_input"], dtype=np.float32))
    t = np.ascontiguousarray(np.asarray(inputs["batch_temporal_mat"], dtype=np.float32))
    Wq = np.asarray(inputs["Wq"], dtype=np.float32)
    Wk = np.asarray(inputs["Wk"], dtype=np.float32)
    Wv = np.asarray(inputs["Wv"], dtype=np.float32)
    Wp = np.asarray(inputs["Wp"], dtype=np.float32)
    Wt1 = np.asarray(inputs["Wt1"], dtype=np.float32)[0]
    Wt2 = np.asarray(inputs["Wt2"], dtype=np.float32)[:, 0]

    C = float(np.sum(Wt2 * np.where(Wt1 >= 0.0, Wt1, 0.2 * Wt1), dtype=np.float64))
    use_bias = abs(C) > 1e-20
    scale = 1.0 / (8.0 * C) if use_bias else 1.0 / 8.0
    wq_s = np.ascontiguousarray(Wq * np.float32(scale))
    wk_c = np.ascontiguousarray(Wk)
    wv_c = np.ascontiguousarray(Wv)
    wp_c = np.ascontiguousarray(Wp)

    nc = _get_nc(C, use_bias)

    in_maps = []
    for b in range(N_CORES):
        in_maps.append({
            "xqT": np.ascontiguousarray(q[b].T),
            "xkT": np.ascontiguousarray(k[b].T),
            "tT": np.ascontiguousarray(t[b].T),
            "wq": wq_s,
            "wk": wk_c,
            "wv": wv_c,
            "wp": wp_c,
        })

    return nc, in_maps


def kernel(**inputs) -> np.ndarray:
    nc, in_maps = prepare(inputs)
    res = run_bass_kernel_spmd(nc, in_maps, list(range(N_CORES)), trace=False)
    return np.stack([res.results[b]["out"] for b in range(N_CORES)], axis=0)
